# revision 8
# baseline (speedup 1.0000x reference)
"""DiT (4-layer, adaLN-modulated transformer) forward on 8 TRN2 NeuronCores.

Sharding: core c -> (batch b=c//2, sequence half c%2), 512 tokens/core.
Activations are feature-major [features, tokens] on chip; x and the output
are transposed on the host. All matmuls are bf16 with f32 PSUM accumulation;
the residual stream stays f32 with deferred SKIP scaling (alpha folding).
adaln is column-sharded across all 8 cores (one AllToAll up front); each
layer exchanges its rope'd K and V halves with a pairwise AllGather.
RoPE is computed in de-interleaved form: the q/k weight columns are permuted
on the host (dot products are permutation-invariant), so rotation pairs sit
in 32-partition blocks and need only elementwise ops.
"""
import sys
import numpy as np

sys.path.insert(0, "/opt/trn_rl_repo")

import ml_dtypes  # noqa: E402
import concourse.bass as bass  # noqa: E402
import concourse.bacc as bacc  # noqa: E402
import concourse.tile as tile  # noqa: E402
from concourse import mybir  # noqa: E402
from concourse.bass_utils import run_bass_kernel_spmd  # noqa: E402

F32 = mybir.dt.float32
BF16 = mybir.dt.bfloat16
AF = mybir.ActivationFunctionType
ALU = mybir.AluOpType
BF = ml_dtypes.bfloat16

D = 1024
NL = 4
H = 16
HD = 64
B = 4
L = 1024
SCALE = HD ** (-0.5)
SKIP = 2.0 ** (-0.5)
EPS = 1e-6

NC = 8          # cores
T = 512         # tokens per core
FT = 8          # feature tiles per 1024 features
KT = 8          # k-tiles of contraction dim D
ADC = 768       # adaln column slice per core (6 tiles of 128)

PAIRS = [[0, 1], [2, 3], [4, 5], [6, 7]]
WORLD = [list(range(NC))]


def _deinterleave_perm():
    p = []
    for h in range(H):
        base = h * HD
        p.extend(base + np.arange(0, HD, 2))
        p.extend(base + np.arange(1, HD, 2))
    return np.array(p, np.int64)


def build(nc, use_vb, use_pb, use_m2b, use_adb):
    x_in = nc.dram_tensor("xt", [128, FT, T], F32, kind="ExternalInput")
    silu_cc = nc.dram_tensor("silu_cc", [128, KT, B], BF16, kind="ExternalInput")
    ropeC = nc.dram_tensor("ropeC", [128, T], F32, kind="ExternalInput")
    ropeS = nc.dram_tensor("ropeS", [128, T], F32, kind="ExternalInput")
    wqk = nc.dram_tensor("wqk", [NL, 4, KT, 128, 512], BF16, kind="ExternalInput")
    wv = nc.dram_tensor("wv", [NL, 2, KT, 128, 512], BF16, kind="ExternalInput")
    wproj = nc.dram_tensor("wproj", [NL, 2, KT, 128, 512], BF16, kind="ExternalInput")
    wm1 = nc.dram_tensor("wm1", [NL, 8, KT, 128, 512], BF16, kind="ExternalInput")
    wm2 = nc.dram_tensor("wm2", [NL, 2, 32, 128, 512], BF16, kind="ExternalInput")
    wad = nc.dram_tensor("wad", [NL, KT, 128, ADC], BF16, kind="ExternalInput")
    bqk = nc.dram_tensor("bqk", [NL, 128, 16], F32, kind="ExternalInput")
    bm1 = nc.dram_tensor("bm1", [NL, 128, 32], F32, kind="ExternalInput")
    vb_b = bpj = bm2 = bad = None
    if use_vb:
        vb_b = nc.dram_tensor("vb_b", [NL, 128, 1024], F32, kind="ExternalInput")
    if use_pb:
        bpj = nc.dram_tensor("bpj", [NL, 128, FT], F32, kind="ExternalInput")
    if use_m2b:
        bm2 = nc.dram_tensor("bm2", [NL, 128, FT], F32, kind="ExternalInput")
    if use_adb:
        bad = nc.dram_tensor("bad", [NL, 128, 48], F32, kind="ExternalInput")
    out = nc.dram_tensor("out", [128, FT, T], F32, kind="ExternalOutput")

    import contextlib
    with tile.TileContext(nc) as tc, contextlib.ExitStack() as ctx:
        # ------------- pools -------------
        singles = ctx.enter_context(tc.tile_pool(name="singles", bufs=1))
        xpool = ctx.enter_context(tc.tile_pool(name="xpool", bufs=1))
        actp = ctx.enter_context(tc.tile_pool(name="actp", bufs=1))
        act2 = ctx.enter_context(tc.tile_pool(name="act2", bufs=2))
        kvp = ctx.enter_context(tc.tile_pool(name="kvp", bufs=1))
        wpool = ctx.enter_context(tc.tile_pool(name="wpool", bufs=6))
        ppool = ctx.enter_context(tc.tile_pool(name="ppool", bufs=4))
        small = ctx.enter_context(tc.tile_pool(name="small", bufs=2))
        scr = ctx.enter_context(tc.tile_pool(name="scr", bufs=3))
        psA = ctx.enter_context(tc.tile_pool(name="psA", bufs=4, space="PSUM"))
        psB = ctx.enter_context(tc.tile_pool(name="psB", bufs=4, space="PSUM"))
        dram = ctx.enter_context(tc.tile_pool(name="dram", bufs=1, space="DRAM"))

        def psa(name):
            return psA.tile([128, 512], F32, name=name, tag="acc")

        def psb(name):
            return psB.tile([128, 512], F32, name=name, tag="sc")

        # ------------- persistent SBUF -------------
        x_sb = xpool.tile([128, FT, T], F32, name="x_sb")
        nc.sync.dma_start(out=x_sb[:], in_=x_in[:])

        cC = singles.tile([128, T], F32, name="cC")
        cS = singles.tile([128, T], F32, name="cS")
        nc.sync.dma_start(out=cC[:], in_=ropeC[:])
        nc.sync.dma_start(out=cS[:], in_=ropeS[:])

        scc = singles.tile([128, KT, B], BF16, name="scc")
        nc.sync.dma_start(out=scc[:], in_=silu_cc[:])

        ones128 = singles.tile([128, 128], BF16, name="ones128")
        nc.vector.memset(ones128[:], 1.0)
        ones64 = ones128[0:1, 0:64]      # K=1 lhsT for denominator broadcast

        bqk_sb = singles.tile([128, NL, 16], F32, name="bqk_sb")
        nc.sync.dma_start(out=bqk_sb[:], in_=bqk.ap().rearrange("l p f -> p l f"))
        bm1_sb = singles.tile([128, NL, 32], F32, name="bm1_sb")
        nc.sync.dma_start(out=bm1_sb[:], in_=bm1.ap().rearrange("l p f -> p l f"))
        vb_sb = bpj_sb = bm2_sb = bad_sb = None
        if use_vb:
            vb_sb = singles.tile([128, NL, 1024], F32, name="vb_sb")
            nc.sync.dma_start(out=vb_sb[:], in_=vb_b.ap().rearrange("l p f -> p l f"))
        if use_pb:
            bpj_sb = singles.tile([128, NL, FT], F32, name="bpj_sb")
            nc.sync.dma_start(out=bpj_sb[:], in_=bpj.ap().rearrange("l p f -> p l f"))
        if use_m2b:
            bm2_sb = singles.tile([128, NL, FT], F32, name="bm2_sb")
            nc.sync.dma_start(out=bm2_sb[:], in_=bm2.ap().rearrange("l p f -> p l f"))
        if use_adb:
            bad_sb = singles.tile([128, NL, 48], F32, name="bad_sb")
            nc.sync.dma_start(out=bad_sb[:], in_=bad.ap().rearrange("l p f -> p l f"))

        # =================================================================
        # adaln, column-sharded: mod[l] = silu(cc) @ adaln_w[l][:, my cols]
        # send layout [dest_core, layer, p, jt] so the post-gather read is
        # partition-major; AllToAll routes batch j//2's slices to core j.
        # =================================================================
        ad_send = dram.tile([NC, NL, 128, 6], F32, name="ad_send")
        ad_gath = dram.tile([NC, NL, 128, 6], F32, name="ad_gath")

        for l in range(NL):
            mod_out = small.tile([128, B, 6], F32, name="mod_out", tag="mod_out")
            for jt in range(6):
                ps = psb(f"mod_ps_{l}_{jt}")
                for k in range(KT):
                    wtile = wpool.tile([128, ADC], BF16, name="wad_t", tag="wad", bufs=2)
                    nc.sync.dma_start(out=wtile[:], in_=wad[l, k])
                    nc.tensor.matmul(
                        ps[0:128, 0:B],
                        lhsT=wtile[:, jt * 128:(jt + 1) * 128],
                        rhs=scc[:, k, :],
                        start=(k == 0), stop=(k == KT - 1))
                nc.vector.tensor_copy(out=mod_out[:, :, jt], in_=ps[0:128, 0:B])
            # send[2b+e, l, p, jt] = mod_out[p, jt, b]  (one DMA per pair slot e)
            for e in range(2):
                src = bass.AP(
                    tensor=mod_out.tensor, offset=mod_out.offset,
                    ap=[list(mod_out.ap[0]), [6, B], [1, 6]])
                dst = bass.AP(
                    tensor=ad_send.tensor,
                    offset=ad_send.offset + l * 128 * 6 + e * NL * 128 * 6,
                    ap=[[6, 128], [2 * NL * 128 * 6, B], [1, 6]])
                nc.gpsimd.dma_start(out=dst, in_=src)

        nc.gpsimd.collective_compute(
            "AllToAll", ALU.bypass,
            ins=[ad_send.opt()], outs=[ad_gath.opt()],
            replica_groups=WORLD)

        # =================================================================
        # layers
        # =================================================================
        kv_send = dram.tile([16, 128, 512], BF16, name="kv_send")
        kv_gath = dram.tile([2, 16, 128, 512], BF16, name="kv_gath")

        alpha = 1.0

        def layernorm_mod(lname, sc_ap, sh_ap, eps_val):
            """h = (LN(x)*(1+sc)+sh)*SKIP as bf16 [128, FT, T].
            sc_ap/sh_ap are [128, 8] parked per-feature vectors, pre-scaled."""
            x16 = actp.tile([128, FT, T], BF16, name=f"x16_{lname}", tag="x16")
            xsq = actp.tile([128, FT, T], BF16, name=f"xsq_{lname}", tag="xsq")
            for ft in range(FT):
                nc.vector.tensor_copy(out=x16[:, ft, :], in_=x_sb[:, ft, :])
                nc.scalar.activation(out=xsq[:, ft, :], in_=x_sb[:, ft, :],
                                     func=AF.Square)
            ps_s = psb(f"ps_sum_{lname}")
            ps_q = psb(f"ps_sq_{lname}")
            for ft in range(FT):
                nc.tensor.matmul(ps_s[:], lhsT=ones128[:], rhs=x16[:, ft, :],
                                 start=(ft == 0), stop=(ft == FT - 1))
            for ft in range(FT):
                nc.tensor.matmul(ps_q[:], lhsT=ones128[:], rhs=xsq[:, ft, :],
                                 start=(ft == 0), stop=(ft == FT - 1))
            mb = small.tile([128, T], F32, name=f"mb_{lname}", tag="mb")
            rb = small.tile([128, T], F32, name=f"rb_{lname}", tag="rb")
            tmp = small.tile([128, T], F32, name=f"tmp_{lname}", tag="lntmp")
            eps_t = small.tile([128, 1], F32, name=f"eps_{lname}", tag="epst")
            nc.vector.memset(eps_t[:], eps_val)
            nc.vector.tensor_scalar_mul(mb[:], ps_s[:], 1.0 / D)
            nc.vector.tensor_scalar_mul(tmp[:], ps_q[:], 1.0 / D)
            nc.vector.tensor_mul(rb[:], mb[:], mb[:])
            nc.vector.tensor_sub(tmp[:], tmp[:], rb[:])
            nc.scalar.activation(out=tmp[:], in_=tmp[:], func=AF.Sqrt, bias=eps_t[:])
            nc.vector.reciprocal(out=rb[:], in_=tmp[:])
            nc.vector.tensor_mul(mb[:], mb[:], rb[:])
            h = act2.tile([128, FT, T], BF16, name=f"h_{lname}", tag="h")
            for ft in range(FT):
                z = scr.tile([128, T], F32, name=f"z_{lname}_{ft}", tag="scratch")
                nc.vector.tensor_mul(z[:], x_sb[:, ft, :], rb[:])
                nc.vector.tensor_sub(z[:], z[:], mb[:])
                nc.scalar.activation(out=h[:, ft, :], in_=z[:], func=AF.Identity,
                                     bias=sh_ap[:, ft:ft + 1],
                                     scale=sc_ap[:, ft:ft + 1])
            return h

        for l in range(NL):
            # ---- mod vectors for this layer's own batch ----
            mod_sb = small.tile([128, 48], F32, name=f"mod_sb_{l}", tag="mod_sb")
            src = bass.AP(
                tensor=ad_gath.tensor, offset=ad_gath.offset + l * 128 * 6,
                ap=[[6, 128], [NL * 128 * 6, NC], [1, 6]])
            nc.sync.dma_start(out=mod_sb[:], in_=src)
            if use_adb:
                nc.vector.tensor_add(mod_sb[:], mod_sb[:], bad_sb[:, l, :])

            park = small.tile([128, 6, FT], F32, name=f"park_{l}", tag="park")
            a_msa = alpha
            a_mlp = alpha * SKIP
            nc.vector.tensor_scalar_mul(park[:, 0, :], mod_sb[:, 0:8], SKIP)
            nc.vector.tensor_scalar(park[:, 1, :], mod_sb[:, 8:16], 1.0, SKIP,
                                    ALU.add, ALU.mult)
            nc.vector.tensor_scalar_mul(park[:, 2, :], mod_sb[:, 16:24], 1.0 / a_msa)
            nc.vector.tensor_scalar_mul(park[:, 3, :], mod_sb[:, 24:32], SKIP)
            nc.vector.tensor_scalar(park[:, 4, :], mod_sb[:, 32:40], 1.0, SKIP,
                                    ALU.add, ALU.mult)
            nc.vector.tensor_scalar_mul(park[:, 5, :], mod_sb[:, 40:48], 1.0 / a_mlp)

            # ======== attention ========
            h = layernorm_mod(f"l{l}a", park[:, 1, :], park[:, 0, :],
                              EPS / (alpha * alpha))

            qk_sb = actp.tile([128, 16, T], BF16, name=f"qk_{l}", tag="qk")
            for g in range(4):
                pss = [psa(f"qk_ps_{l}_{g}_{i}") for i in range(4)]
                for k in range(KT):
                    wtile = wpool.tile([128, 512], BF16, name="wqk_t", tag="w")
                    nc.sync.dma_start(out=wtile[:], in_=wqk[l, g, k])
                    for i in range(4):
                        nc.tensor.matmul(
                            pss[i][:], lhsT=wtile[:, i * 128:(i + 1) * 128],
                            rhs=h[:, k, :], start=(k == 0), stop=(k == KT - 1))
                for i in range(4):
                    ft = g * 4 + i
                    nc.scalar.activation(out=qk_sb[:, ft, :], in_=pss[i][:],
                                         func=AF.Identity,
                                         bias=bqk_sb[:, l, ft:ft + 1])

            vloc = actp.tile([128, 4, 1024], BF16, name=f"vloc_{l}", tag="vloc")
            for g in range(2):
                pss = [psa(f"v_ps_{l}_{g}_{i}") for i in range(4)]
                for k in range(KT):
                    wtile = wpool.tile([128, 512], BF16, name="wv_t", tag="w")
                    nc.sync.dma_start(out=wtile[:], in_=wv[l, g, k])
                    for i in range(4):
                        nc.tensor.matmul(
                            pss[i][:], lhsT=h[:, k, i * 128:(i + 1) * 128],
                            rhs=wtile[:], start=(k == 0), stop=(k == KT - 1))
                for i in range(4):
                    nc.vector.tensor_copy(out=vloc[:, i, g * 512:(g + 1) * 512],
                                          in_=pss[i][:])
            if use_vb:
                for i in range(4):
                    nc.vector.tensor_add(vloc[:, i, :], vloc[:, i, :], vb_sb[:, l, :])

            # ---- rope on q (ft 0..7) and k (ft 8..15) ----
            rq = actp.tile([128, 16, T], BF16, name=f"rq_{l}", tag="rq")
            for ft in range(16):
                swp = scr.tile([128, T], F32, name=f"swp_{l}_{ft}", tag="scratch")
                t1 = scr.tile([128, T], F32, name=f"t1_{l}_{ft}", tag="scratch")
                for blk in range(4):
                    s = blk * 32
                    os_ = (blk ^ 1) * 32
                    sgn = -1.0 if blk % 2 == 0 else 1.0
                    nc.vector.tensor_scalar_mul(swp[s:s + 32, :],
                                                qk_sb[os_:os_ + 32, ft, :], sgn)
                nc.vector.tensor_mul(swp[:], swp[:], cS[:])
                nc.vector.tensor_mul(t1[:], qk_sb[:, ft, :], cC[:])
                nc.vector.tensor_add(t1[:], t1[:], swp[:])
                nc.vector.tensor_copy(out=rq[:, ft, :], in_=t1[:])

            # ---- exchange rope'd k and v within the pair ----
            for ft in range(8):
                nc.gpsimd.dma_start(out=kv_send[ft], in_=rq[:, 8 + ft, :])
            for i in range(4):
                for hc in range(2):
                    nc.gpsimd.dma_start(out=kv_send[8 + 2 * i + hc],
                                        in_=vloc[:, i, hc * 512:(hc + 1) * 512])
            nc.gpsimd.collective_compute(
                "AllGather", ALU.bypass,
                ins=[kv_send.opt()], outs=[kv_gath.opt()],
                replica_groups=PAIRS)

            kfull = kvp.tile([128, FT, 1024], BF16, name=f"kfull_{l}", tag="kfull")
            v_sb = kvp.tile([128, 8, 16, 65], BF16, name=f"v_sb_{l}", tag="v_sb")
            nc.vector.memset(v_sb[:, :, :, 64:65], 1.0)
            for half in range(2):
                for ft in range(8):
                    nc.sync.dma_start(
                        out=kfull[:, ft, half * 512:(half + 1) * 512],
                        in_=kv_gath[half, ft])
                for i in range(4):
                    for hc in range(2):
                        src = kv_gath[half, 8 + 2 * i + hc] \
                            .rearrange("p (h c) -> p h c", h=8)
                        nc.sync.dma_start(
                            out=v_sb[:, half * 4 + i, hc * 8:hc * 8 + 8, 0:64],
                            in_=src)

            # ---- attention: S^T = k^T.T @ q^T per head, exp, [v|1] matmul ----
            o_sb = actp.tile([128, FT, T], BF16, name=f"o_{l}", tag="o")
            for pr in range(8):
                accs = []
                for sub in range(2):
                    hh = 2 * pr + sub
                    po = sub * 64
                    acc = psA.tile([128, 512], F32, name=f"att_acc_{l}_{hh}",
                                   tag="acc")
                    accs.append(acc)
                    for j in range(8):
                        ps = psb(f"sc_ps_{l}_{hh}_{j}")
                        nc.tensor.matmul(
                            ps[:], lhsT=kfull[po:po + 64, pr, j * 128:(j + 1) * 128],
                            rhs=rq[po:po + 64, pr, :], start=True, stop=True)
                        pexp = ppool.tile([128, 512], BF16, name=f"pexp_{l}",
                                          tag="pexp")
                        nc.scalar.activation(out=pexp[:], in_=ps[:], func=AF.Exp,
                                             scale=SCALE)
                        nc.tensor.matmul(acc[0:65, :], lhsT=v_sb[:, j, hh, :],
                                         rhs=pexp[:], start=(j == 0), stop=(j == 7))
                for sub in range(2):
                    hh = 2 * pr + sub
                    rd = small.tile([1, 512], BF16, name=f"rd_{l}_{hh}", tag="rd")
                    with nc.allow_low_precision(reason="softmax denom bf16 for PE"):
                        nc.vector.reciprocal(out=rd[:], in_=accs[sub][64:65, :])
                    dps = psb(f"den_ps_{l}_{hh}")
                    nc.tensor.matmul(dps[0:64, :], lhsT=ones64, rhs=rd[:],
                                     start=True, stop=True)
                    dn_sb = ppool.tile([64, 512], BF16, name=f"dn_{l}_{hh}",
                                       tag="dn")
                    nc.scalar.activation(out=dn_sb[:], in_=dps[0:64, :],
                                         func=AF.Copy)
                    nc.vector.tensor_mul(o_sb[sub * 64:sub * 64 + 64, pr, :],
                                         accs[sub][0:64, :], dn_sb[:])

            # ---- proj + residual ----
            for g in range(2):
                pss = [psa(f"pj_ps_{l}_{g}_{i}") for i in range(4)]
                for k in range(KT):
                    wtile = wpool.tile([128, 512], BF16, name="wpj_t", tag="w")
                    nc.sync.dma_start(out=wtile[:], in_=wproj[l, g, k])
                    for i in range(4):
                        nc.tensor.matmul(
                            pss[i][:], lhsT=wtile[:, i * 128:(i + 1) * 128],
                            rhs=o_sb[:, k, :], start=(k == 0), stop=(k == KT - 1))
                for i in range(4):
                    ft = g * 4 + i
                    nc.vector.scalar_tensor_tensor(
                        out=x_sb[:, ft, :], in0=pss[i][:],
                        scalar=park[:, 2, ft:ft + 1], in1=x_sb[:, ft, :],
                        op0=ALU.mult, op1=ALU.add)
                    if use_pb:
                        gb = small.tile([128, 1], F32, name=f"gbp_{l}_{ft}", tag="gb")
                        nc.vector.tensor_mul(gb[:], park[:, 2, ft:ft + 1],
                                             bpj_sb[:, l, ft:ft + 1])
                        nc.vector.tensor_scalar_add(x_sb[:, ft, :], x_sb[:, ft, :],
                                                    gb[:])
            alpha *= SKIP

            # ======== mlp ========
            h2 = layernorm_mod(f"l{l}m", park[:, 4, :], park[:, 3, :],
                               EPS / (alpha * alpha))

            m1_sb = actp.tile([128, 32, T], BF16, name=f"m1_{l}", tag="m1")
            for g in range(8):
                pss = [psa(f"m1_ps_{l}_{g}_{i}") for i in range(4)]
                for k in range(KT):
                    wtile = wpool.tile([128, 512], BF16, name="wm1_t", tag="w")
                    nc.sync.dma_start(out=wtile[:], in_=wm1[l, g, k])
                    for i in range(4):
                        nc.tensor.matmul(
                            pss[i][:], lhsT=wtile[:, i * 128:(i + 1) * 128],
                            rhs=h2[:, k, :], start=(k == 0), stop=(k == KT - 1))
                for i in range(4):
                    mt = g * 4 + i
                    nc.scalar.activation(out=m1_sb[:, mt, :], in_=pss[i][:],
                                         func=AF.Gelu_apprx_tanh,
                                         bias=bm1_sb[:, l, mt:mt + 1])

            for g in range(2):
                pss = [psa(f"m2_ps_{l}_{g}_{i}") for i in range(4)]
                for k in range(32):
                    wtile = wpool.tile([128, 512], BF16, name="wm2_t", tag="w")
                    nc.sync.dma_start(out=wtile[:], in_=wm2[l, g, k])
                    for i in range(4):
                        nc.tensor.matmul(
                            pss[i][:], lhsT=wtile[:, i * 128:(i + 1) * 128],
                            rhs=m1_sb[:, k, :], start=(k == 0), stop=(k == 31))
                for i in range(4):
                    ft = g * 4 + i
                    nc.vector.scalar_tensor_tensor(
                        out=x_sb[:, ft, :], in0=pss[i][:],
                        scalar=park[:, 5, ft:ft + 1], in1=x_sb[:, ft, :],
                        op0=ALU.mult, op1=ALU.add)
                    if use_m2b:
                        gb = small.tile([128, 1], F32, name=f"gbm_{l}_{ft}", tag="gb")
                        nc.vector.tensor_mul(gb[:], park[:, 5, ft:ft + 1],
                                             bm2_sb[:, l, ft:ft + 1])
                        nc.vector.tensor_scalar_add(x_sb[:, ft, :], x_sb[:, ft, :],
                                                    gb[:])
            alpha *= SKIP

        # final deferred scale + store
        for ft in range(FT):
            xo = scr.tile([128, T], F32, name=f"xo_{ft}", tag="scratch")
            nc.scalar.activation(out=xo[:], in_=x_sb[:, ft, :], func=AF.Copy,
                                 scale=alpha)
            nc.sync.dma_start(out=out.ap()[:, ft, :], in_=xo[:])
    return nc


def _pack_inputs(inputs):
    x = np.asarray(inputs["x"], np.float32)
    c = np.asarray(inputs["c"], np.float32)
    t = np.asarray(inputs["t"], np.float32)
    qkv_w = np.asarray(inputs["qkv_w"], np.float32)
    qkv_b = np.asarray(inputs["qkv_b"], np.float32)
    proj_w = np.asarray(inputs["proj_w"], np.float32)
    proj_b = np.asarray(inputs["proj_b"], np.float32)
    mlp_w1 = np.asarray(inputs["mlp_w1"], np.float32)
    mlp_b1 = np.asarray(inputs["mlp_b1"], np.float32)
    mlp_w2 = np.asarray(inputs["mlp_w2"], np.float32)
    mlp_b2 = np.asarray(inputs["mlp_b2"], np.float32)
    adaln_w = np.asarray(inputs["adaln_w"], np.float32)
    adaln_b = np.asarray(inputs["adaln_b"], np.float32)

    perm = _deinterleave_perm()
    wq = qkv_w[:, :, 0:D][:, :, perm]
    wk = qkv_w[:, :, D:2 * D][:, :, perm]
    wqk = np.concatenate([wq, wk], axis=2)                       # [NL, D, 2D]
    # pack [..., K, 128, groups, 512] -> [NL, groups, K, 128, 512]
    wqk_pack = np.ascontiguousarray(
        wqk.reshape(NL, KT, 128, 4, 512).transpose(0, 3, 1, 2, 4)).astype(BF)
    wv_pack = np.ascontiguousarray(
        qkv_w[:, :, 2 * D:].reshape(NL, KT, 128, 2, 512)
        .transpose(0, 3, 1, 2, 4)).astype(BF)
    wpj_pack = np.ascontiguousarray(
        proj_w.reshape(NL, KT, 128, 2, 512).transpose(0, 3, 1, 2, 4)).astype(BF)
    wm1_pack = np.ascontiguousarray(
        mlp_w1.reshape(NL, KT, 128, 8, 512).transpose(0, 3, 1, 2, 4)).astype(BF)
    wm2_pack = np.ascontiguousarray(
        mlp_w2.reshape(NL, 32, 128, 2, 512).transpose(0, 3, 1, 2, 4)).astype(BF)

    bqk_v = np.concatenate([qkv_b[:, 0:D][:, perm],
                            qkv_b[:, D:2 * D][:, perm]], 1)
    bqk_pack = np.ascontiguousarray(
        bqk_v.reshape(NL, 16, 128).transpose(0, 2, 1)).astype(np.float32)
    bm1_pack = np.ascontiguousarray(
        mlp_b1.reshape(NL, 32, 128).transpose(0, 2, 1)).astype(np.float32)
    vb = qkv_b[:, 2 * D:]
    use_vb = bool(np.any(vb != 0))
    use_pb = bool(np.any(proj_b != 0))
    use_m2b = bool(np.any(mlp_b2 != 0))
    use_adb = bool(np.any(adaln_b != 0))

    pos = np.arange(L, dtype=np.float32)
    omega = 1.0 / (10000.0 ** (np.arange(0, HD, 2, dtype=np.float32) / HD))
    ang = pos[:, None] * omega[None, :]
    cosT = np.cos(ang).T.astype(np.float32)                      # [32, L]
    sinT = np.sin(ang).T.astype(np.float32)

    cc = (c[:, 0, :] + t) * SKIP                                 # [B, D]
    silu_cc = (cc / (1.0 + np.exp(-cc))).astype(np.float32)
    scc_pack = np.ascontiguousarray(
        silu_cc.T.reshape(KT, 128, B).transpose(1, 0, 2)).astype(BF)

    per_core = []
    for cid in range(NC):
        b, half = cid // 2, cid % 2
        l0 = half * T
        xt = x[b, l0:l0 + T, :].T                                # [D, T]
        xt_pack = np.ascontiguousarray(
            xt.reshape(FT, 128, T).transpose(1, 0, 2)).astype(np.float32)
        m = {
            "xt": xt_pack,
            "silu_cc": scc_pack,
            "ropeC": np.ascontiguousarray(np.tile(cosT[:, l0:l0 + T], (4, 1))),
            "ropeS": np.ascontiguousarray(np.tile(sinT[:, l0:l0 + T], (4, 1))),
            "wqk": wqk_pack, "wv": wv_pack, "wproj": wpj_pack,
            "wm1": wm1_pack, "wm2": wm2_pack,
            "wad": np.ascontiguousarray(
                adaln_w[:, :, cid * ADC:(cid + 1) * ADC]
                .reshape(NL, KT, 128, ADC)).astype(BF),
            "bqk": bqk_pack, "bm1": bm1_pack,
        }
        if use_vb:
            m["vb_b"] = np.ascontiguousarray(
                np.broadcast_to(vb[:, None, :], (NL, 128, 1024))).astype(np.float32)
        if use_pb:
            m["bpj"] = np.ascontiguousarray(
                proj_b.reshape(NL, FT, 128).transpose(0, 2, 1)).astype(np.float32)
        if use_m2b:
            m["bm2"] = np.ascontiguousarray(
                mlp_b2.reshape(NL, FT, 128).transpose(0, 2, 1)).astype(np.float32)
        if use_adb:
            m["bad"] = np.ascontiguousarray(
                adaln_b.reshape(NL, 48, 128).transpose(0, 2, 1)).astype(np.float32)
        per_core.append(m)
    return per_core, (use_vb, use_pb, use_m2b, use_adb)


_CACHE = {}


def _get_nc(flags):
    if flags not in _CACHE:
        nc = bacc.Bacc("TRN2", target_bir_lowering=False, debug=False,
                       num_devices=NC)
        build(nc, *flags)
        nc.compile()
        _CACHE[flags] = nc
    return _CACHE[flags]


def kernel(**inputs) -> np.ndarray:
    in_maps, flags = _pack_inputs(inputs)
    nc = _get_nc(flags)
    res = run_bass_kernel_spmd(nc, in_maps, core_ids=list(range(NC)))
    full = np.zeros((B, L, D), np.float32)
    for cid in range(NC):
        b, half = cid // 2, cid % 2
        l0 = half * T
        o = np.asarray(res.results[cid]["out"])                  # [128, FT, T]
        full[b, l0:l0 + T, :] = o.transpose(1, 0, 2).reshape(D, T).T
    return full
